# revision 2
# baseline (speedup 1.0000x reference)
"""BeaconGPT Trainium2 kernel: 8-core SPMD, sequence-sharded with KV AllGather.

Sharding strategy:
- 2048 tokens = 16 blocks of 128. Core c owns blocks {c, 15-c} (balanced causal work).
- All weights replicated (bf16); activations sequence-sharded, f32 residual.
- Per layer: local rmsnorm -> qkv (act-stationary matmuls) -> qk-norm + rope
  (even/odd-permuted features) -> one AllGather of bf16 K^T,V -> causal
  attention (k-major scores, no-max softmax since |scores|<=8, fused ones-row
  denominator) -> out-proj -> squared-relu MLP. Uniform SPMD control flow:
  per-core causality is handled by per-core mask *data* (k-loop bounds 8/16).
- Final: rmsnorm -> AllGather x^T -> vocab-sharded lm_head (6284 cols/core).

kernel(**inputs) takes full inputs, returns full [1, 2048, 50272] f32 logits.
"""
import os
import sys

import numpy as np
import ml_dtypes

for _p in ("/opt/trn_rl_repo", "/root/.axon_site/_ro/trn_rl_repo"):
    if os.path.isdir(_p) and _p not in sys.path:
        sys.path.append(_p)

import concourse.bass as bass
import concourse.bacc as bacc
import concourse.mybir as mybir
import concourse.tile as tile
from concourse.bass_utils import run_bass_kernel_spmd
from concourse.masks import make_identity

BF = ml_dtypes.bfloat16
FP32 = mybir.dt.float32
BF16 = mybir.dt.bfloat16
I32 = mybir.dt.int32
AFT = mybir.ActivationFunctionType
ALU = mybir.AluOpType
AX = mybir.AxisListType

P = 128
NCORES = 8
SEQ, D, NH, HD, FFN = 2048, 1024, 16, 64, 4096
NHP = NH // 2          # 8 head pairs
NL = int(os.environ.get("BEACON_NL", "8"))
NLP = max(NL, 1)       # param dim (avoid zero-sized inputs)
VOCAB = 50272
VS = VOCAB // NCORES   # 6284
VSP = 6400             # padded vocab shard (50 x 128)
NB = SEQ // P          # 16 blocks
EPS = 1e-5
KB0 = 8                # uniform k-block bound for q-tile 0 (blocks 0..7)
PHASES = os.environ.get("BEACON_PHASES", "ABCDEFG")
EP = int(os.environ.get("BEACON_EP", "3"))
VT_SIZES = [512] * 12 + [256]          # 6400 = 12*512 + 256
VT_GROUPS = [(i, min(i + 2, 13)) for i in range(0, 13, 2)]  # groups of 2


def block_loc(b):
    """global block -> (owning rank, local tile)"""
    return (b, 0) if b < 8 else (15 - b, 1)


def build_program():
    nc = bacc.Bacc("TRN2", target_bir_lowering=False, debug=False,
                   num_devices=NCORES)

    ids_d = nc.dram_tensor("ids", [P, 2], I32, kind="ExternalInput").ap()
    wte_d = nc.dram_tensor("wte", [VOCAB, D], FP32, kind="ExternalInput").ap()
    wqkvT_d = nc.dram_tensor("wqkvT", [NLP, D, 3 * D], BF16, kind="ExternalInput").ap()
    woT_d = nc.dram_tensor("woT", [NLP, D, D], BF16, kind="ExternalInput").ap()
    w1T_d = nc.dram_tensor("w1T", [NLP, D, FFN], BF16, kind="ExternalInput").ap()
    w2T_d = nc.dram_tensor("w2T", [NLP, FFN, D], BF16, kind="ExternalInput").ap()
    lmT_d = nc.dram_tensor("lmT", [D, VSP], BF16, kind="ExternalInput").ap()
    cos_d = nc.dram_tensor("cos", [P, 2, 32], FP32, kind="ExternalInput").ap()
    sin_d = nc.dram_tensor("sin", [P, 2, 32], FP32, kind="ExternalInput").ap()
    # masks: [P, 8*256 (kb 0..7, q-tile0|q-tile1) + 8*128 (kb 8..15, q-tile1)]
    masks_d = nc.dram_tensor("masks", [P, 3072], BF16, kind="ExternalInput").ap()
    out_d = nc.dram_tensor("out", [SEQ, VSP], FP32, kind="ExternalOutput").ap()

    with tile.TileContext(nc) as tc:
        with (
            tc.tile_pool(name="const", bufs=1) as const,
            tc.tile_pool(name="xp", bufs=1) as xp,
            tc.tile_pool(name="big", bufs=1) as big,
            tc.tile_pool(name="wk", bufs=1) as wk,
            tc.tile_pool(name="dramp", bufs=2, space="DRAM") as dramp,
            tc.tile_pool(name="ps", bufs=1, space="PSUM") as ps,
        ):
            # ---- constants
            ident = const.tile([P, P], BF16)
            make_identity(nc, ident)
            eps_t = const.tile([P, 1], FP32)
            nc.vector.memset(eps_t[:], EPS)
            ones1 = const.tile([1, 64], BF16)
            nc.vector.memset(ones1[:], 1.0)
            masks_sb = const.tile([P, 3072], BF16)
            nc.sync.dma_start(masks_sb[:], masks_d[:])
            m07 = masks_sb[:, 0:2048].rearrange("p (kb q) -> p kb q", kb=8)
            m815 = masks_sb[:, 2048:3072].rearrange("p (kb q) -> p kb q", kb=8)
            cos_sb = const.tile([P, 2, 32], FP32)
            nc.sync.dma_start(cos_sb[:], cos_d[:])
            sin_sb = const.tile([P, 2, 32], FP32)
            nc.sync.dma_start(sin_sb[:], sin_d[:])
            ids_sb = const.tile([P, 2], I32)
            nc.sync.dma_start(ids_sb[:], ids_d[:])

            x_sb = xp.tile([P, 2, D], FP32)

            def rmsnorm_into(src, dst, nfeat):
                """src [P, N] f32 -> dst = src * rsqrt(mean(src^2)+eps)"""
                sq = wk.tile([P, nfeat], FP32, tag="sq", bufs=2, name="sq")
                nc.vector.tensor_mul(sq[:], src, src)
                red = wk.tile([P, 1], FP32, tag="red", bufs=2, name="red")
                nc.vector.tensor_reduce(red[:], sq[:], axis=AX.X, op=ALU.add)
                nc.scalar.activation(red[:], red[:], AFT.Sqrt,
                                     bias=eps_t[:, 0:1], scale=1.0 / nfeat)
                nc.vector.reciprocal(red[:], red[:])
                nc.vector.tensor_scalar_mul(dst, src, red[:])

            def transpose128(src_bf, dst_bf):
                """[128,128] bf16 SBUF -> PE transpose -> dst [128,128] bf16"""
                tp = ps.tile([P, P], BF16, tag="mm512", bufs=3, name="tp")
                nc.tensor.transpose(tp[:], src_bf, ident[:])
                nc.any.tensor_copy(dst_bf, tp[:])

            # ---- embedding: x = rmsnorm(wte[ids])
            for t in range(2):
                emb = wk.tile([P, D], FP32, tag="f32_1k", bufs=2, name="emb")
                nc.gpsimd.indirect_dma_start(
                    out=emb[:], out_offset=None, in_=wte_d[:],
                    in_offset=bass.IndirectOffsetOnAxis(ap=ids_sb[:, t:t + 1], axis=0),
                )
                rmsnorm_into(emb[:], x_sb[:, t, :], D)

            # ---- transformer layers
            for l in range(NL):
                if "A" not in PHASES:
                    continue
                # Phase A: h = rmsnorm(x) -> bf16 -> hT [P, 8, 256]
                h_bf = wk.tile([P, 2, D], BF16, tag="h_bf", name="h_bf")
                for t in range(2):
                    rmsnorm_into(x_sb[:, t, :], h_bf[:, t, :], D)
                hT = wk.tile([P, 8, 256], BF16, tag="hT", name="hT")
                for t in range(2):
                    for j in range(8):
                        transpose128(h_bf[:, t, j * P:(j + 1) * P],
                                     hT[:, j, t * P:(t + 1) * P])

                if "B" not in PHASES:
                    continue
                # Phase B: qkv = h @ wqkv^T  -> [P, 2, 3072] bf16
                qkv = wk.tile([P, 2, 3 * D], BF16, tag="qkv", name="qkv")
                for ft in range(6):
                    pst = [ps.tile([P, 512], FP32, tag="mm512", bufs=3,
                                   name=f"qkv_ps{_t}") for _t in range(2)]
                    for j in range(8):
                        wt = wk.tile([P, 512], BF16, tag="w512", bufs=3, name="wt")
                        nc.sync.dma_start(
                            wt[:], wqkvT_d[l, j * P:(j + 1) * P,
                                           ft * 512:(ft + 1) * 512])
                        for t in range(2):
                            nc.tensor.matmul(
                                pst[t][:], lhsT=hT[:, j, t * P:(t + 1) * P],
                                rhs=wt[:], start=(j == 0), stop=(j == 7))
                    for t in range(2):
                        nc.any.tensor_copy(qkv[:, t, ft * 512:(ft + 1) * 512],
                                           pst[t][:])

                if "C" not in PHASES:
                    continue
                # Phase C: qk-norm + rope (in-place on qkv); v used as-is
                for t in range(2):
                    for src_off in (0, D):
                        src = qkv[:, t, src_off:src_off + D]
                        srch = src.rearrange("p (h d) -> p h d", h=NH)
                        sq = wk.tile([P, D], FP32, tag="sq", bufs=2, name="sq2")
                        nc.vector.tensor_mul(sq[:], src, src)
                        r16 = wk.tile([P, NH], FP32, tag="red16", bufs=2, name="r16")
                        nc.vector.tensor_reduce(
                            r16[:], sq[:].rearrange("p (h d) -> p h d", h=NH),
                            axis=AX.X, op=ALU.add)
                        nc.scalar.activation(r16[:], r16[:], AFT.Sqrt,
                                             bias=eps_t[:, 0:1], scale=1.0 / HD)
                        nc.vector.reciprocal(r16[:], r16[:])
                        qn = wk.tile([P, NH, HD], FP32, tag="f32_1k", bufs=2,
                                     name="qn")
                        nc.vector.tensor_tensor(
                            qn[:], srch,
                            r16[:, :, None].to_broadcast([P, NH, HD]), ALU.mult)
                        # rope on [evens|odds] layout, write back into qkv (bf16)
                        e_ = qn[:, :, 0:32]
                        o_ = qn[:, :, 32:64]
                        cb = cos_sb[:, t, None, :].to_broadcast([P, NH, 32])
                        sb_ = sin_sb[:, t, None, :].to_broadcast([P, NH, 32])
                        t1 = wk.tile([P, NH, 32], FP32, tag="rope1", bufs=2,
                                     name="t1")
                        t2 = wk.tile([P, NH, 32], FP32, tag="rope2", bufs=2,
                                     name="t2")
                        nc.vector.tensor_tensor(t1[:], o_, cb, ALU.mult)
                        nc.vector.tensor_tensor(t2[:], e_, sb_, ALU.mult)
                        nc.vector.tensor_sub(srch[:, :, 0:32], t1[:], t2[:])
                        nc.vector.tensor_tensor(t1[:], o_, sb_, ALU.mult)
                        nc.vector.tensor_tensor(t2[:], e_, cb, ALU.mult)
                        nc.vector.tensor_add(srch[:, :, 32:64], t1[:], t2[:])

                if "D" not in PHASES:
                    continue
                # Phase D: local transposes -> qT, kTl; contribute; AllGather
                qT = wk.tile([P, 8, 256], BF16, tag="qT", name="qT")
                kTl = wk.tile([P, 8, 256], BF16, tag="kTl", name="kTl")
                for t in range(2):
                    for hp in range(NHP):
                        transpose128(qkv[:, t, hp * P:(hp + 1) * P],
                                     qT[:, hp, t * P:(t + 1) * P])
                        transpose128(qkv[:, t, D + hp * P:D + (hp + 1) * P],
                                     kTl[:, hp, t * P:(t + 1) * P])
                contrib = dramp.tile([2, P, 2048], BF16, tag="contrib",
                                     name="contrib")
                nc.sync.dma_start(contrib[0], kTl[:].rearrange("p a b -> p (a b)"))
                nc.sync.dma_start(contrib[1].rearrange("p (a b) -> p a b", a=2),
                                  qkv[:, :, 2 * D:3 * D])
                gathered = dramp.tile([NCORES, 2, P, 2048], BF16,
                                      addr_space="Shared", tag="gath", name="gath")
                nc.gpsimd.collective_compute(
                    "AllGather", ALU.bypass,
                    replica_groups=[list(range(NCORES))],
                    ins=[contrib.opt()], outs=[gathered.opt()],
                )

                # stage gathered V (+ones col) into SBUF; K^T streamed per use
                v_all = big.tile([P, NB, NH, HD + 1], BF16, tag="vbig",
                                 name="v_all")
                for b in range(NB):
                    r, tt = block_loc(b)
                    src_v = gathered[r, 1].rearrange(
                        "p (two h d) -> p two h d", two=2, d=HD)[:, tt, :, :]
                    nc.sync.dma_start(v_all[:, b, :, 0:HD], src_v)
                nc.vector.memset(v_all[:, :, :, HD:HD + 1], 1.0)

                if "E" not in PHASES:
                    continue
                # Phase E: attention (k-major scores, fused ones-row denominator)
                attn_oT = wk.tile([P, NHP, 256], BF16, tag="attn_oT",
                                  name="attn_oT")
                for hp in range(NHP):
                    av = [[ps.tile([65, P], FP32, tag="av", bufs=4,
                                   name=f"av{_h}{_t}")
                           for _t in range(2)] for _h in range(2)]
                    for kb in range(NB):
                        both = kb < KB0
                        qn_ = 256 if both else 128
                        qoff = 0 if both else 128
                        r, tt = block_loc(kb)
                        ktc = wk.tile([P, P], BF16, tag="ktc", bufs=4, name="ktc")
                        nc.sync.dma_start(
                            ktc[:], gathered[r, 0].rearrange(
                                "p (hp two q) -> p hp two q", two=2, q=P)[:, hp, tt, :])
                        es = wk.tile([P, 2, 256], BF16, tag="es", bufs=3, name="es")
                        for h in range(2):
                            sch = ps.tile([P, 256], FP32, tag="mm512", bufs=3,
                                          name="sch")
                            nc.tensor.matmul(
                                sch[:, 0:qn_],
                                lhsT=ktc[h * 64:(h + 1) * 64, :],
                                rhs=qT[h * 64:(h + 1) * 64, hp, qoff:qoff + qn_],
                                start=True, stop=True)
                            nc.scalar.activation(es[:, h, qoff:qoff + qn_],
                                                 sch[:, 0:qn_],
                                                 AFT.Exp, scale=0.125)
                        if EP >= 1:
                            if both:
                                nc.vector.tensor_tensor(
                                    es[:, :, :], es[:, :, :],
                                    m07[:, kb, None, :].to_broadcast([P, 2, 256]),
                                    ALU.mult)
                            else:
                                nc.vector.tensor_tensor(
                                    es[:, :, 128:256], es[:, :, 128:256],
                                    m815[:, kb - 8, None, :].to_broadcast([P, 2, 128]),
                                    ALU.mult)
                        if EP >= 2:
                            for h in range(2):
                                for t in range(2):
                                    if t == 0 and not both:
                                        continue
                                    nc.tensor.matmul(
                                        av[h][t][:],
                                        lhsT=v_all[:, kb, 2 * hp + h, :],
                                        rhs=es[:, h, t * P:(t + 1) * P],
                                        start=(kb == 0),
                                        stop=(kb == (KB0 - 1 if t == 0 else NB - 1)))
                    if EP < 3:
                        continue
                    # evict: normalize by fused denominator (row 64)
                    den = wk.tile([1, 512], FP32, tag="den", bufs=2, name="den")
                    for h in range(2):
                        for t in range(2):
                            nc.vector.tensor_copy(
                                den[0:1, (2 * h + t) * P:(2 * h + t + 1) * P],
                                av[h][t][64:65, :])
                    nc.vector.reciprocal(den[:], den[:])
                    den_bf = wk.tile([1, 512], BF16, tag="den_bf", bufs=2,
                                     name="den_bf")
                    nc.vector.tensor_copy(den_bf[:], den[:])
                    bc = ps.tile([64, 512], FP32, tag="bc", bufs=1, name="bc")
                    nc.tensor.matmul(bc[:], lhsT=ones1[:], rhs=den_bf[:],
                                     start=True, stop=True)
                    bc_sb = wk.tile([64, 512], FP32, tag="bc_sb", bufs=2,
                                    name="bc_sb")
                    nc.any.tensor_copy(bc_sb[:], bc[:])
                    for h in range(2):
                        for t in range(2):
                            nc.vector.tensor_mul(
                                attn_oT[h * 64:(h + 1) * 64, hp, t * P:(t + 1) * P],
                                av[h][t][0:64, :],
                                bc_sb[:, (2 * h + t) * P:(2 * h + t + 1) * P])

                if "F" not in PHASES:
                    continue
                # Phase F: x += attn_out @ wo^T
                for half in range(2):
                    pst = [ps.tile([P, 512], FP32, tag="mm512", bufs=3,
                                   name=f"wo_ps{_t}") for _t in range(2)]
                    for j in range(8):
                        wt = wk.tile([P, 512], BF16, tag="w512", bufs=3, name="wt2")
                        nc.sync.dma_start(
                            wt[:], woT_d[l, j * P:(j + 1) * P,
                                         half * 512:(half + 1) * 512])
                        for t in range(2):
                            nc.tensor.matmul(
                                pst[t][:], lhsT=attn_oT[:, j, t * P:(t + 1) * P],
                                rhs=wt[:], start=(j == 0), stop=(j == 7))
                    for t in range(2):
                        xs = x_sb[:, t, half * 512:(half + 1) * 512]
                        nc.vector.tensor_add(xs, pst[t][:], xs)

                if "G" not in PHASES:
                    continue
                # Phase G: MLP. h2 = rmsnorm(x); mlpT = relu(h2@w1^T)^2 (transposed)
                h2_bf = wk.tile([P, 2, D], BF16, tag="h_bf", name="h2_bf")
                for t in range(2):
                    rmsnorm_into(x_sb[:, t, :], h2_bf[:, t, :], D)
                h2T = wk.tile([P, 8, 256], BF16, tag="hT", name="h2T")
                for t in range(2):
                    for j in range(8):
                        transpose128(h2_bf[:, t, j * P:(j + 1) * P],
                                     h2T[:, j, t * P:(t + 1) * P])
                mlpT = wk.tile([P, 32, 256], BF16, tag="mlpT", name="mlpT")
                for fc in range(32):
                    slab = wk.tile([P, 8, P], BF16, tag="w1slab", bufs=3,
                                   name="slab")
                    nc.sync.dma_start(
                        slab[:], w1T_d[l][:, fc * P:(fc + 1) * P].rearrange(
                            "(j p) f -> p j f", p=P))
                    psw = ps.tile([P, 256], FP32, tag="mm512", bufs=3, name="psw")
                    for j in range(8):
                        nc.tensor.matmul(psw[:], lhsT=slab[:, j, :],
                                         rhs=h2T[:, j, :],
                                         start=(j == 0), stop=(j == 7))
                    tmp = wk.tile([P, 256], FP32, tag="relu_t", bufs=3, name="tmp")
                    nc.vector.tensor_scalar_max(tmp[:], psw[:], 0.0)
                    nc.scalar.activation(mlpT[:, fc, :], tmp[:], AFT.Square)
                # x += mlp @ w2^T
                for fh in range(2):
                    pst = [ps.tile([P, 512], FP32, tag="mm512", bufs=3,
                                   name=f"w2_ps{_t}") for _t in range(2)]
                    for fc in range(32):
                        wt = wk.tile([P, 512], BF16, tag="w512", bufs=3, name="wt3")
                        nc.sync.dma_start(
                            wt[:], w2T_d[l, fc * P:(fc + 1) * P,
                                         fh * 512:(fh + 1) * 512])
                        for t in range(2):
                            nc.tensor.matmul(
                                pst[t][:], lhsT=mlpT[:, fc, t * P:(t + 1) * P],
                                rhs=wt[:], start=(fc == 0), stop=(fc == 31))
                    for t in range(2):
                        xs = x_sb[:, t, fh * 512:(fh + 1) * 512]
                        nc.vector.tensor_add(xs, pst[t][:], xs)

            # ---- final norm, AllGather x^T, lm_head
            xf_bf = wk.tile([P, 2, D], BF16, tag="h_bf", name="xf_bf")
            for t in range(2):
                rmsnorm_into(x_sb[:, t, :], xf_bf[:, t, :], D)
            xfT = wk.tile([P, 8, 256], BF16, tag="hT", name="xfT")
            for t in range(2):
                for j in range(8):
                    transpose128(xf_bf[:, t, j * P:(j + 1) * P],
                                 xfT[:, j, t * P:(t + 1) * P])
            contrib_x = dramp.tile([P, 2048], BF16, tag="contribx", name="contribx")
            nc.sync.dma_start(contrib_x[:], xfT[:].rearrange("p a b -> p (a b)"))
            gathered_x = dramp.tile([NCORES, P, 2048], BF16,
                                    addr_space="Shared", tag="gathx", name="gathx")
            nc.gpsimd.collective_compute(
                "AllGather", ALU.bypass,
                replica_groups=[list(range(NCORES))],
                ins=[contrib_x.opt()], outs=[gathered_x.opt()],
            )
            xfT_all = big.tile([P, 8, NB, P], BF16, tag="vbig", name="xfT_all")
            for b in range(NB):
                r, tt = block_loc(b)
                nc.sync.dma_start(
                    xfT_all[:, :, b, :],
                    gathered_x[r].rearrange("p (hp two q) -> p hp two q",
                                            two=2, q=P)[:, :, tt, :])

            for vs_, ve_ in VT_GROUPS:
                ng = ve_ - vs_
                slab2 = wk.tile([P, 8, 2, 512], BF16, tag="lmslab", bufs=2,
                                name="slab2")
                for i, vt in enumerate(range(vs_, ve_)):
                    n = VT_SIZES[vt]
                    nc.sync.dma_start(
                        slab2[:, :, i, 0:n],
                        lmT_d[:, vt * 512:vt * 512 + n].rearrange(
                            "(j p) f -> p j f", p=P))
                for b in range(NB):
                    pss = [ps.tile([P, 512], FP32, tag="mm512", bufs=3,
                                   name=f"lmps{_i}") for _i in range(ng)]
                    for j in range(8):
                        for i, vt in enumerate(range(vs_, ve_)):
                            n = VT_SIZES[vt]
                            nc.tensor.matmul(
                                pss[i][:, 0:n], lhsT=xfT_all[:, j, b, :],
                                rhs=slab2[:, j, i, 0:n],
                                start=(j == 0), stop=(j == 7))
                    for i, vt in enumerate(range(vs_, ve_)):
                        n = VT_SIZES[vt]
                        ot = wk.tile([P, 512], FP32, tag="lmout", bufs=3,
                                     name="ot")
                        nc.any.tensor_copy(ot[:, 0:n], pss[i][:, 0:n])
                        nc.sync.dma_start(
                            out_d[b * P:(b + 1) * P, vt * 512:vt * 512 + n],
                            ot[:, 0:n])

    nc.compile()
    return nc


# ---------------- host side ----------------

_CACHE = {}


def _perm_evenodd():
    p = []
    for h in range(NH):
        base = h * HD
        p += list(range(base, base + HD, 2))
        p += list(range(base + 1, base + HD, 2))
    return np.array(p)


def _prep_inputs(inputs):
    """Build per-core in_maps from full inputs."""
    ids_full = np.asarray(inputs["input_ids"]).astype(np.int32).reshape(SEQ)
    wte = np.asarray(inputs["wte"], dtype=np.float32)
    wqkv = np.asarray(inputs["wqkv"], dtype=np.float32)
    wo = np.asarray(inputs["wo"], dtype=np.float32)
    w1 = np.asarray(inputs["w1"], dtype=np.float32)
    w2 = np.asarray(inputs["w2"], dtype=np.float32)
    lm_head = np.asarray(inputs["lm_head"], dtype=np.float32)

    perm = _perm_evenodd()
    wqkvT = np.zeros((NLP, D, 3 * D), dtype=BF)
    woT = np.zeros((NLP, D, D), dtype=BF)
    w1T = np.zeros((NLP, D, FFN), dtype=BF)
    w2T = np.zeros((NLP, FFN, D), dtype=BF)
    for l in range(NL):
        wq = wqkv[l][:D][perm]
        wk_ = wqkv[l][D:2 * D][perm]
        wv = wqkv[l][2 * D:]
        wqkvT[l] = np.concatenate([wq, wk_, wv], axis=0).T.astype(BF)
        woT[l] = wo[l].T.astype(BF)
        w1T[l] = w1[l].T.astype(BF)
        w2T[l] = w2[l].T.astype(BF)

    inv_freq = 1.0 / (10000.0 ** (np.arange(0, HD, 2, dtype=np.float32) / HD))
    tril = np.tril(np.ones((P, P), np.float32))  # tril[q, k]: q >= k

    in_maps = []
    for c in range(NCORES):
        blocks = (c, 15 - c)
        ids_c = np.stack([ids_full[b * P:(b + 1) * P] for b in blocks], axis=1)
        pos = np.stack(
            [np.arange(b * P, (b + 1) * P, dtype=np.float32) for b in blocks],
            axis=1)  # [P, 2]
        ang = pos[:, :, None] * inv_freq[None, None, :]  # [P, 2, 32]

        g0, g1 = blocks
        # masks in scoresT orientation: mask[k_local, q_local] = 1 if valid
        mk07 = np.zeros((P, 8, 256), np.float32)
        for kb in range(8):
            if kb < g0:
                mk07[:, kb, 0:P] = 1.0
            elif kb == g0:
                mk07[:, kb, 0:P] = tril.T  # [k, q]: valid iff q >= k
            mk07[:, kb, P:256] = 1.0  # tile1 block g1 >= 8 > kb: always valid
        mk815 = np.zeros((P, 8, 128), np.float32)
        for kb in range(8, 16):
            if kb < g1:
                mk815[:, kb - 8] = 1.0
            elif kb == g1:
                mk815[:, kb - 8] = tril.T
        masks = np.concatenate(
            [mk07.reshape(P, 2048), mk815.reshape(P, 1024)], axis=1).astype(BF)

        lmT_c = np.zeros((D, VSP), dtype=BF)
        lmT_c[:, :VS] = lm_head[c * VS:(c + 1) * VS].T.astype(BF)

        in_maps.append({
            "ids": np.ascontiguousarray(ids_c),
            "wte": wte,
            "wqkvT": wqkvT,
            "woT": woT,
            "w1T": w1T,
            "w2T": w2T,
            "lmT": lmT_c,
            "cos": np.cos(ang).astype(np.float32),
            "sin": np.sin(ang).astype(np.float32),
            "masks": masks,
        })
    return in_maps


def run(inputs, trace=False, tmpdir=None):
    """Returns (full_logits [1, SEQ, VOCAB] f32, BassKernelResults)."""
    if "nc" not in _CACHE:
        _CACHE["nc"] = build_program()
    nc = _CACHE["nc"]
    in_maps = _prep_inputs(inputs)
    kw = {}
    if trace:
        kw = dict(trace=True)
        if tmpdir is not None:
            kw["tmpdir"] = tmpdir
    res = run_bass_kernel_spmd(nc, in_maps, core_ids=list(range(NCORES)), **kw)
    full = np.empty((SEQ, VOCAB), np.float32)
    for c in range(NCORES):
        full[:, c * VS:(c + 1) * VS] = res.results[c]["out"][:, :VS]
    return full.reshape(1, SEQ, VOCAB), res


def kernel(**inputs):
    out, _ = run(inputs)
    return out


def bench(inputs, iters=10, warmup=3):
    """Time device execution with device-resident inputs (no H2D in the loop).

    Returns (per_iter_seconds_list, outputs_core0_sample). Replicates
    bass2jax.run_bass_via_pjrt's multi-core shard_map structure, minus
    donation and minus host->device transfer inside the timed region.
    """
    import time as _time
    import jax
    from jax.experimental.shard_map import shard_map
    from jax.sharding import Mesh, PartitionSpec, NamedSharding
    from concourse import bass2jax as b2j
    from concourse import mybir as _mybir

    if "nc" not in _CACHE:
        _CACHE["nc"] = build_program()
    nc = _CACHE["nc"]
    in_maps = _prep_inputs(inputs)

    b2j.install_neuronx_cc_hook()
    partition_name = (nc.partition_id_tensor.name
                      if nc.partition_id_tensor else None)
    in_names, out_names, out_avals, zero_outs = [], [], [], []
    for alloc in nc.m.functions[0].allocations:
        if not isinstance(alloc, _mybir.MemoryLocationSet):
            continue
        name = alloc.memorylocations[0].name
        if alloc.kind == "ExternalInput":
            if name != partition_name:
                in_names.append(name)
        elif alloc.kind == "ExternalOutput":
            out_names.append(name)
            shape = tuple(alloc.tensor_shape)
            dtype = _mybir.dt.np(alloc.dtype)
            out_avals.append(jax.core.ShapedArray(shape, dtype))
            zero_outs.append(np.zeros(shape, dtype))
    n_params = len(in_names)
    all_in_names = list(in_names) + list(out_names)
    if partition_name is not None:
        all_in_names.append(partition_name)

    def _body(*args):
        operands = list(args)
        if partition_name is not None:
            operands.append(b2j.partition_id_tensor())
        outs = b2j._bass_exec_p.bind(
            *operands,
            out_avals=tuple(out_avals),
            in_names=tuple(all_in_names),
            out_names=tuple(out_names),
            lowering_input_output_aliases=(),
            sim_require_finite=True,
            sim_require_nnan=True,
            nc=nc,
        )
        return tuple(outs)

    devices = jax.devices()[:NCORES]
    mesh = Mesh(np.asarray(devices), ("core",))
    n_outs = len(out_names)
    in_specs = (PartitionSpec("core"),) * (n_params + n_outs)
    out_specs = (PartitionSpec("core"),) * n_outs
    fn = jax.jit(shard_map(_body, mesh=mesh, in_specs=in_specs,
                           out_specs=out_specs, check_rep=False))

    sharding = NamedSharding(mesh, PartitionSpec("core"))
    dev_args = []
    for i, name in enumerate(in_names):
        concat = np.concatenate([np.asarray(m[name]) for m in in_maps], axis=0)
        dev_args.append(jax.device_put(concat, sharding))
    for z in zero_outs:
        concat = np.zeros((NCORES * z.shape[0], *z.shape[1:]), z.dtype)
        dev_args.append(jax.device_put(concat, sharding))

    for _ in range(warmup):
        outs = fn(*dev_args)
        jax.block_until_ready(outs)
    times = []
    for _ in range(iters):
        t0 = _time.perf_counter()
        outs = fn(*dev_args)
        jax.block_until_ready(outs)
        times.append(_time.perf_counter() - t0)
    out0 = np.asarray(outs[0]).reshape(NCORES, *out_avals[0].shape)
    return times, out0



# revision 38
# speedup vs baseline: 1.2939x; 1.2939x over previous
"""BeaconGPT Trainium2 kernel: 8-core SPMD, sequence-sharded with KV AllGather.

Sharding strategy:
- 2048 tokens = 16 blocks of 128. Core c owns blocks {c, 15-c} (balanced causal work).
- All weights replicated (bf16); activations sequence-sharded, f32 residual.
- Per layer: local rmsnorm -> qkv (act-stationary matmuls) -> qk-norm + rope
  (even/odd-permuted features) -> one AllGather of bf16 K^T,V -> causal
  attention (k-major scores, no-max softmax since |scores|<=8, fused ones-row
  denominator) -> out-proj -> squared-relu MLP. Uniform SPMD control flow:
  per-core causality is handled by per-core mask *data* (k-loop bounds 8/16).
- Final: rmsnorm -> AllGather x^T -> vocab-sharded lm_head (6284 cols/core).

kernel(**inputs) takes full inputs, returns full [1, 2048, 50272] f32 logits.
"""
import os
import sys

import numpy as np
import ml_dtypes

for _p in ("/opt/trn_rl_repo", "/root/.axon_site/_ro/trn_rl_repo"):
    if os.path.isdir(_p) and _p not in sys.path:
        sys.path.append(_p)

import concourse.bass as bass
import concourse.bacc as bacc
import concourse.mybir as mybir
import concourse.tile as tile
from concourse.bass_utils import run_bass_kernel_spmd
from concourse.masks import make_identity

BF = ml_dtypes.bfloat16
F8NP = ml_dtypes.float8_e4m3
FP32 = mybir.dt.float32
BF16 = mybir.dt.bfloat16
F8 = mybir.dt.float8e4
I32 = mybir.dt.int32
LM_SCALE = 256.0       # lm_head fp8 weight pre-scale (values ~U(-0.01,0.01))
QKV_SCALE = 64.0       # wqkv fp8 pre-scale (values ~U(-0.027, 0.027))
W1_SCALE = 64.0        # w1 fp8 pre-scale
W2_SCALE = 2048.0      # w2 fp8 pre-scale (values ~U(-0.0027, 0.0027))
AFT = mybir.ActivationFunctionType
ALU = mybir.AluOpType
AX = mybir.AxisListType

P = 128
NCORES = 8
SEQ, D, NH, HD, FFN = 2048, 1024, 16, 64, 4096
NHP = NH // 2          # 8 head pairs
NL = int(os.environ.get("BEACON_NL", "8"))
NLP = max(NL, 1)       # param dim (avoid zero-sized inputs)
VOCAB = 50272
VS = VOCAB // NCORES   # 6284
VSP = 6400             # padded vocab shard (50 x 128)
NB = SEQ // P          # 16 blocks
EPS = 1e-5
KB0 = 8                # uniform k-block bound for q-tile 0 (blocks 0..7)
PHASES = os.environ.get("BEACON_PHASES", "ABCDEFG")
EP = int(os.environ.get("BEACON_EP", "3"))
VT_SIZES = [512] * 12 + [256]          # 6400 = 12*512 + 256
VT_GROUPS = [(i, min(i + 2, 13)) for i in range(0, 13, 2)]  # groups of 2


def block_loc(b):
    """global block -> (owning rank, local tile)"""
    return (b, 0) if b < 8 else (15 - b, 1)


def build_program():
    nc = bacc.Bacc("TRN2", target_bir_lowering=False, debug=False,
                   num_devices=NCORES)

    ids_d = nc.dram_tensor("ids", [P, 2], I32, kind="ExternalInput").ap()
    wte_d = nc.dram_tensor("wte", [VOCAB, D], FP32, kind="ExternalInput").ap()
    wqkvT_d = nc.dram_tensor("wqkvT", [NLP, D, 3 * D], BF16, kind="ExternalInput").ap()
    woT_d = nc.dram_tensor("woT", [NLP, D, D], BF16, kind="ExternalInput").ap()
    w1T_d = nc.dram_tensor("w1T", [NLP, D, FFN], BF16, kind="ExternalInput").ap()
    w2T_d = nc.dram_tensor("w2T", [NLP, FFN, D], BF16, kind="ExternalInput").ap()
    lmT_d = nc.dram_tensor("lmT", [D, VSP], BF16, kind="ExternalInput").ap()
    cos_d = nc.dram_tensor("cos", [P, 2, 32], FP32, kind="ExternalInput").ap()
    sin_d = nc.dram_tensor("sin", [P, 2, 32], FP32, kind="ExternalInput").ap()
    # masks: [P, 8*256 (kb 0..7, q-tile0|q-tile1) + 8*128 (kb 8..15, q-tile1)]
    masks_d = nc.dram_tensor("masks", [P, 3072], BF16, kind="ExternalInput").ap()
    out_d = nc.dram_tensor("out", [SEQ, VSP], FP32, kind="ExternalOutput").ap()

    with tile.TileContext(nc) as tc:
        with (
            tc.tile_pool(name="const", bufs=1) as const,
            tc.tile_pool(name="xp", bufs=1) as xp,
            tc.tile_pool(name="big", bufs=1) as big,
            tc.tile_pool(name="wk", bufs=1) as wk,
            tc.tile_pool(name="dramp", bufs=2, space="DRAM") as dramp,
            tc.tile_pool(name="ps", bufs=1, space="PSUM") as ps,
        ):
            # ---- constants
            ident = const.tile([P, P], BF16)
            make_identity(nc, ident)
            eps_t = const.tile([P, 1], FP32)
            nc.vector.memset(eps_t[:], EPS)
            ones1 = const.tile([1, 64], BF16)
            nc.vector.memset(ones1[:], 1.0)
            ones_col = const.tile([P, 1], BF16)
            nc.vector.memset(ones_col[:], 1.0)
            masks_sb = const.tile([P, 3072], BF16)
            nc.sync.dma_start(masks_sb[:], masks_d[:])
            m07 = masks_sb[:, 0:2048].rearrange("p (kb q) -> p kb q", kb=8)
            m815 = masks_sb[:, 2048:3072].rearrange("p (kb q) -> p kb q", kb=8)
            cos_sb = const.tile([P, 2, 32], FP32)
            nc.sync.dma_start(cos_sb[:], cos_d[:])
            sin_sb = const.tile([P, 2, 32], FP32)
            nc.sync.dma_start(sin_sb[:], sin_d[:])
            ids_sb = const.tile([P, 2], I32)
            nc.sync.dma_start(ids_sb[:], ids_d[:])

            x_sb = xp.tile([P, 2, D], FP32)
            # V-with-ones staging tile: ones column interleaved once, V data
            # refreshed per layer; contributed contiguously to the AllGather
            v_ones = big.tile([P, 2, NH, HD + 1], BF16, tag="vones",
                              name="v_ones")
            nc.vector.memset(v_ones[:, :, :, HD:HD + 1], 1.0)

            def rmsnorm_into(src, dst, nfeat):
                """src [P, N] f32 -> dst = src * rsqrt(mean(src^2)+eps)"""
                sq = wk.tile([P, nfeat], FP32, tag="sq", bufs=1, name="sq")
                red = wk.tile([P, 1], FP32, tag="red", bufs=2, name="red")
                nc.vector.scalar_tensor_tensor(
                    sq[:], src, 1.0, src, op0=ALU.mult, op1=ALU.mult,
                    accum_out=red[:])
                nc.scalar.activation(red[:], red[:], AFT.Sqrt,
                                     bias=eps_t[:, 0:1], scale=1.0 / nfeat)
                nc.vector.reciprocal(red[:], red[:])
                nc.vector.tensor_scalar_mul(dst, src, red[:])

            def transpose128(src_bf, dst_bf):
                """[128,128] bf16 SBUF -> PE transpose -> dst [128,128] bf16"""
                tp = ps.tile([P, P], BF16, tag="mm512", bufs=4, name="tp")
                nc.tensor.transpose(tp[:], src_bf, ident[:])
                nc.vector.tensor_copy(dst_bf, tp[:])

            # ---- embedding: x = rmsnorm(wte[ids])
            for t in range(2):
                emb = wk.tile([P, D], FP32, tag="f32_1k", bufs=2, name="emb")
                nc.gpsimd.indirect_dma_start(
                    out=emb[:], out_offset=None, in_=wte_d[:],
                    in_offset=bass.IndirectOffsetOnAxis(ap=ids_sb[:, t:t + 1], axis=0),
                )
                rmsnorm_into(emb[:], x_sb[:, t, :], D)

            # ---- transformer layers
            for l in range(NL):
                if "A" not in PHASES:
                    continue
                # Phase A: h = rmsnorm(x) -> bf16 -> hT [P, 8, 256] (fp8)
                h_bf = wk.tile([P, 2, D], BF16, tag="h_bf", name="h_bf")
                for t in range(2):
                    rmsnorm_into(x_sb[:, t, :], h_bf[:, t, :], D)
                hT = wk.tile([P, 8, 256], BF16, tag="hT", name="hT")
                for t in range(2):
                    for j in range(8):
                        transpose128(h_bf[:, t, j * P:(j + 1) * P],
                                     hT[:, j, t * P:(t + 1) * P])

                if "B" not in PHASES:
                    continue
                # Phase B: qkv = h @ wqkv^T -> [P, 2, 3072] bf16.
                # V columns first so the V AllGather launches while K/Q still
                # compute; K next (rope+transpose+K AllGather), Q last.
                qkv = wk.tile([P, 2, 3 * D], BF16, tag="qkv", name="qkv")

                def qkv_ft(ft):
                    wt = wk.tile([P, 8, 512], BF16, tag="w8x512", bufs=2,
                                 name="wt")
                    nc.sync.dma_start(
                        wt[:], wqkvT_d[l][:, ft * 512:(ft + 1) * 512].rearrange(
                            "(j p) f -> p j f", p=P))
                    pst = [ps.tile([P, 512], FP32, tag="mm512", bufs=4,
                                   name=f"qkv_ps{_t}") for _t in range(2)]
                    for j in range(8):
                        for t in range(2):
                            nc.tensor.matmul(
                                pst[t][:], lhsT=hT[:, j, t * P:(t + 1) * P],
                                rhs=wt[:, j, :], start=(j == 0), stop=(j == 7))
                    for t in range(2):
                        nc.vector.tensor_copy(qkv[:, t, ft * 512:(ft + 1) * 512],
                                              pst[t][:])

                qkv_ft(4)
                qkv_ft(5)
                # V AllGather: V third of qkv with the ones column interleaved
                nc.vector.tensor_copy(
                    v_ones[:, :, :, 0:HD],
                    qkv[:, :, 2 * D:3 * D].rearrange("p t (h d) -> p t h d",
                                                     d=HD))
                VW = 2 * NH * (HD + 1)  # 2080
                contribV = dramp.tile([P, VW], BF16, tag="contribV",
                                      name="contribV")
                nc.sync.dma_start(
                    contribV[:], v_ones[:].rearrange("p t h d -> p (t h d)"))
                gatheredV = dramp.tile([NCORES, P, VW], BF16,
                                       addr_space="Shared", tag="gathV",
                                       name="gathV")
                nc.gpsimd.collective_compute(
                    "AllGather", ALU.bypass,
                    replica_groups=[list(range(NCORES))],
                    ins=[contribV.opt()], outs=[gatheredV.opt()],
                )
                qkv_ft(2)
                qkv_ft(3)

                if "C" not in PHASES:
                    continue

                # Phase C: qk-norm + rope (in-place on qkv); v used as-is.
                # K first so the AllGather can launch before Q is processed.
                def norm_rope(src_off):
                    for t in range(2):
                        src = qkv[:, t, src_off:src_off + D]
                        srch = src.rearrange("p (h d) -> p h d", h=NH)
                        sq = wk.tile([P, D], FP32, tag="sq", bufs=1, name="sq2")
                        nc.vector.tensor_mul(sq[:], src, src)
                        r16 = wk.tile([P, NH], FP32, tag="red16", bufs=2,
                                      name="r16")
                        nc.vector.tensor_reduce(
                            r16[:], sq[:].rearrange("p (h d) -> p h d", h=NH),
                            axis=AX.X, op=ALU.add)
                        nc.scalar.activation(r16[:], r16[:], AFT.Sqrt,
                                             bias=eps_t[:, 0:1], scale=1.0 / HD)
                        nc.vector.reciprocal(r16[:], r16[:])
                        qn = wk.tile([P, NH, HD], FP32, tag="f32_1k", bufs=2,
                                     name="qn")
                        nc.vector.tensor_tensor(
                            qn[:], srch,
                            r16[:, :, None].to_broadcast([P, NH, HD]), ALU.mult)
                        # rope on [evens|odds] layout, write back into qkv (bf16)
                        e_ = qn[:, :, 0:32]
                        o_ = qn[:, :, 32:64]
                        cb = cos_sb[:, t, None, :].to_broadcast([P, NH, 32])
                        sb_ = sin_sb[:, t, None, :].to_broadcast([P, NH, 32])
                        t1 = wk.tile([P, NH, 32], FP32, tag="rope1", bufs=2,
                                     name="t1")
                        t2 = wk.tile([P, NH, 32], FP32, tag="rope2", bufs=2,
                                     name="t2")
                        nc.vector.tensor_tensor(t1[:], o_, cb, ALU.mult)
                        nc.vector.tensor_tensor(t2[:], e_, sb_, ALU.mult)
                        nc.vector.tensor_sub(srch[:, :, 0:32], t1[:], t2[:])
                        nc.vector.tensor_tensor(t1[:], o_, sb_, ALU.mult)
                        nc.vector.tensor_tensor(t2[:], e_, cb, ALU.mult)
                        nc.vector.tensor_add(srch[:, :, 32:64], t1[:], t2[:])

                norm_rope(D)  # K

                if "D" not in PHASES:
                    continue
                # Phase D: K transposes -> kTl; K AllGather launches while
                # Q qkv-matmuls + rope + transposes still run.
                qT = wk.tile([P, 8, 256], BF16, tag="qT", name="qT")
                kTl = wk.tile([P, 8, 256], BF16, tag="kTl", name="kTl")
                for t in range(2):
                    for hp in range(NHP):
                        transpose128(qkv[:, t, D + hp * P:D + (hp + 1) * P],
                                     kTl[:, hp, t * P:(t + 1) * P])
                contribK = dramp.tile([P, 2048], BF16, tag="contribK",
                                      name="contribK")
                nc.sync.dma_start(contribK[:], kTl[:].rearrange("p a b -> p (a b)"))
                gatheredK = dramp.tile([NCORES, P, 2048], BF16,
                                       addr_space="Shared", tag="gathK",
                                       name="gathK")
                nc.gpsimd.collective_compute(
                    "AllGather", ALU.bypass,
                    replica_groups=[list(range(NCORES))],
                    ins=[contribK.opt()], outs=[gatheredK.opt()],
                )

                qkv_ft(0)
                qkv_ft(1)
                norm_rope(0)  # Q (overlaps the K AllGather)
                for t in range(2):
                    for hp in range(NHP):
                        transpose128(qkv[:, t, hp * P:(hp + 1) * P],
                                     qT[:, hp, t * P:(t + 1) * P])

                # stage gathered V into SBUF: one fully-contiguous DMA per
                # rank ([P, 2080] both sides; trivial descriptors)
                v_all = big.tile([P, NCORES, 2, NH, HD + 1], BF16, tag="vbig",
                                 name="v_all")
                for r in range(NCORES):
                    nc.sync.dma_start(
                        v_all[:, r].rearrange("p two h d -> p (two h d)"),
                        gatheredV[r])

                if "E" not in PHASES:
                    continue
                # Phase E: attention (k-major scores, fused ones-row denominator)
                attn_oT = wk.tile([P, NHP, 256], BF16, tag="attn_oT",
                                  name="attn_oT")
                for hp in range(NHP):
                    # all 16 K^T blocks for this head pair in one DMA
                    kthp = wk.tile([P, NCORES, 2, P], BF16, tag="kthp", bufs=2,
                                   name="kthp")
                    if os.environ.get("BEACON_KTHP", "1") == "1":
                        nc.sync.dma_start(
                            kthp[:], gatheredK.rearrange(
                                "r p (hp two q) -> p hp r two q",
                                two=2, q=P)[:, hp])
                    else:
                        for rr in range(NCORES):
                            nc.sync.dma_start(
                                kthp[:, rr, :, :], gatheredK[rr].rearrange(
                                    "p (hp two q) -> p hp two q",
                                    two=2, q=P)[:, hp])
                    av = [[ps.tile([65, P], FP32, tag="av", bufs=4,
                                   name=f"av{_h}{_t}")
                           for _t in range(2)] for _h in range(2)]
                    for kb in range(NB):
                        both = kb < KB0
                        qn_ = 256 if both else 128
                        qoff = 0 if both else 128
                        r, tt = block_loc(kb)
                        es = wk.tile([P, 2, 256], BF16, tag="es", bufs=3, name="es")
                        for h in range(2):
                            sch = ps.tile([P, 256], FP32, tag="mm512", bufs=4,
                                          name="sch")
                            nc.tensor.matmul(
                                sch[:, 0:qn_],
                                lhsT=kthp[h * 64:(h + 1) * 64, r, tt, :],
                                rhs=qT[h * 64:(h + 1) * 64, hp, qoff:qoff + qn_],
                                start=True, stop=True)
                            nc.scalar.activation(es[:, h, qoff:qoff + qn_],
                                                 sch[:, 0:qn_],
                                                 AFT.Exp, scale=0.125)
                        if EP >= 1:
                            if both:
                                # q-tile1 (cols 128:256) is always valid for
                                # kb<8; only tile0 needs the mask
                                nc.vector.tensor_tensor(
                                    es[:, :, 0:128], es[:, :, 0:128],
                                    m07[:, kb, None, 0:128].to_broadcast(
                                        [P, 2, 128]),
                                    ALU.mult)
                            else:
                                nc.vector.tensor_tensor(
                                    es[:, :, 128:256], es[:, :, 128:256],
                                    m815[:, kb - 8, None, :].to_broadcast(
                                        [P, 2, 128]),
                                    ALU.mult)
                        if EP >= 2:
                            for h in range(2):
                                for t in range(2):
                                    if t == 0 and not both:
                                        continue
                                    nc.tensor.matmul(
                                        av[h][t][:],
                                        lhsT=v_all[:, r, tt, 2 * hp + h, :],
                                        rhs=es[:, h, t * P:(t + 1) * P],
                                        start=(kb == 0),
                                        stop=(kb == (KB0 - 1 if t == 0 else NB - 1)))
                    if EP < 3:
                        continue
                    # evict: normalize by fused denominator (row 64)
                    den = wk.tile([1, 512], FP32, tag="den", bufs=2, name="den")
                    for h in range(2):
                        for t in range(2):
                            nc.vector.tensor_copy(
                                den[0:1, (2 * h + t) * P:(2 * h + t + 1) * P],
                                av[h][t][64:65, :])
                    nc.vector.reciprocal_approx_fast(den[:], den[:])
                    den_bf = wk.tile([1, 512], BF16, tag="den_bf", bufs=2,
                                     name="den_bf")
                    nc.vector.tensor_copy(den_bf[:], den[:])
                    bc = ps.tile([64, 512], FP32, tag="mm512", bufs=4, name="bc")
                    nc.tensor.matmul(bc[:], lhsT=ones1[:], rhs=den_bf[:],
                                     start=True, stop=True)
                    bc_sb = wk.tile([64, 512], FP32, tag="bc_sb", bufs=2,
                                    name="bc_sb")
                    nc.vector.tensor_copy(bc_sb[:], bc[:])
                    for h in range(2):
                        for t in range(2):
                            nc.vector.tensor_mul(
                                attn_oT[h * 64:(h + 1) * 64, hp, t * P:(t + 1) * P],
                                av[h][t][0:64, :],
                                bc_sb[:, (2 * h + t) * P:(2 * h + t + 1) * P])

                if "F" not in PHASES:
                    continue
                # Phase F: x += attn_out @ wo^T
                for half in range(2):
                    wt = wk.tile([P, 8, 512], BF16, tag="w8x512", bufs=2,
                                 name="wt2")
                    nc.sync.dma_start(
                        wt[:], woT_d[l][:, half * 512:(half + 1) * 512].rearrange(
                            "(j p) f -> p j f", p=P))
                    pst = [ps.tile([P, 512], FP32, tag="mm512", bufs=4,
                                   name=f"wo_ps{_t}") for _t in range(2)]
                    for j in range(8):
                        for t in range(2):
                            nc.tensor.matmul(
                                pst[t][:], lhsT=attn_oT[:, j, t * P:(t + 1) * P],
                                rhs=wt[:, j, :], start=(j == 0), stop=(j == 7))
                    for t in range(2):
                        xs = x_sb[:, t, half * 512:(half + 1) * 512]
                        nc.vector.tensor_add(xs, pst[t][:], xs)

                if "G" not in PHASES:
                    continue
                # Phase G: MLP. h2 = rmsnorm(x); mlpT = relu(h2@w1^T)^2 (transposed)
                h2_bf = wk.tile([P, 2, D], BF16, tag="h_bf", name="h2_bf")
                for t in range(2):
                    rmsnorm_into(x_sb[:, t, :], h2_bf[:, t, :], D)
                h2T = wk.tile([P, 8, 256], BF16, tag="hT", name="h2T")
                for t in range(2):
                    for j in range(8):
                        transpose128(h2_bf[:, t, j * P:(j + 1) * P],
                                     h2T[:, j, t * P:(t + 1) * P])
                mlpT = wk.tile([P, 32, 256], BF16, tag="mlpT", name="mlpT")
                for g in range(8):
                    slab = wk.tile([P, 8, 512], BF16, tag="w8x512", bufs=2,
                                   name="slab")
                    nc.sync.dma_start(
                        slab[:], w1T_d[l][:, g * 512:(g + 1) * 512].rearrange(
                            "(j p) f -> p j f", p=P))
                    for i in range(4):
                        fc = g * 4 + i
                        psw = ps.tile([P, 256], FP32, tag="mm512", bufs=4,
                                      name="psw")
                        for j in range(8):
                            nc.tensor.matmul(
                                psw[:], lhsT=slab[:, j, i * P:(i + 1) * P],
                                rhs=h2T[:, j, :], start=(j == 0), stop=(j == 7))
                        tmp = wk.tile([P, 256], FP32, tag="relu_t", bufs=3,
                                      name="tmp")
                        nc.vector.tensor_scalar_max(tmp[:], psw[:], 0.0)
                        nc.vector.tensor_mul(mlpT[:, fc, :], tmp[:], tmp[:])
                # x += mlp @ w2^T
                for fh in range(2):
                    pst = [ps.tile([P, 512], FP32, tag="mm512", bufs=4,
                                   name=f"w2_ps{_t}") for _t in range(2)]
                    for fc8 in range(4):
                        wt = wk.tile([P, 8, 512], BF16, tag="w8x512", bufs=2,
                                     name="wt3")
                        nc.sync.dma_start(
                            wt[:], w2T_d[l, fc8 * 1024:(fc8 + 1) * 1024,
                                         fh * 512:(fh + 1) * 512].rearrange(
                                             "(j p) f -> p j f", p=P))
                        for j8 in range(8):
                            fc = fc8 * 8 + j8
                            for t in range(2):
                                nc.tensor.matmul(
                                    pst[t][:],
                                    lhsT=mlpT[:, fc, t * P:(t + 1) * P],
                                    rhs=wt[:, j8, :], start=(fc == 0),
                                    stop=(fc == 31))
                    for t in range(2):
                        xs = x_sb[:, t, fh * 512:(fh + 1) * 512]
                        nc.vector.tensor_add(xs, pst[t][:], xs)

            # ---- final norm, AllGather x^T (fp8), fp8 DoubleRow lm_head
            xf_bf = wk.tile([P, 2, D], BF16, tag="h_bf", name="xf_bf")
            for t in range(2):
                rmsnorm_into(x_sb[:, t, :], xf_bf[:, t, :], D)
            xfT = wk.tile([P, 8, 256], BF16, tag="hT", name="xfT")
            for t in range(2):
                for j in range(8):
                    tp = ps.tile([P, P], BF16, tag="mm512", bufs=4, name="tp")
                    nc.tensor.transpose(tp[:], xf_bf[:, t, j * P:(j + 1) * P],
                                        ident[:])
                    nc.vector.tensor_copy(xfT[:, j, t * P:(t + 1) * P], tp[:])
            contrib_x = dramp.tile([P, 2048], BF16, tag="contribx", name="contribx")
            nc.sync.dma_start(contrib_x[:], xfT[:].rearrange("p a b -> p (a b)"))
            gathered_x = dramp.tile([NCORES, P, 2048], BF16,
                                    addr_space="Shared", tag="gathx", name="gathx")
            nc.gpsimd.collective_compute(
                "AllGather", ALU.bypass,
                replica_groups=[list(range(NCORES))],
                ins=[contrib_x.opt()], outs=[gathered_x.opt()],
            )
            xfT_all = big.tile([P, 8, NB, P], BF16, tag="vbig", name="xfT_all")
            for b in range(NB):
                r, tt = block_loc(b)
                nc.sync.dma_start(
                    xfT_all[:, :, b, :],
                    gathered_x[r].rearrange("p (hp two q) -> p hp two q",
                                            two=2, q=P)[:, :, tt, :])

            for vs_, ve_ in VT_GROUPS:
                ng = ve_ - vs_
                slab2 = wk.tile([P, 8, 2, 512], BF16, tag="lmslab", bufs=2,
                                name="slab2")
                for i, vt in enumerate(range(vs_, ve_)):
                    n = VT_SIZES[vt]
                    nc.sync.dma_start(
                        slab2[:, :, i, 0:n],
                        lmT_d[:, vt * 512:vt * 512 + n].rearrange(
                            "(j p) f -> p j f", p=P))
                for b in range(NB):
                    pss = [ps.tile([P, 512], FP32, tag="mm512", bufs=4,
                                   name=f"lmps{_i}") for _i in range(ng)]
                    for j in range(8):
                        for i, vt in enumerate(range(vs_, ve_)):
                            n = VT_SIZES[vt]
                            nc.tensor.matmul(
                                pss[i][:, 0:n], lhsT=xfT_all[:, j, b, :],
                                rhs=slab2[:, j, i, 0:n],
                                start=(j == 0), stop=(j == 7))
                    for i, vt in enumerate(range(vs_, ve_)):
                        n = VT_SIZES[vt]
                        ot = wk.tile([P, 512], FP32, tag="lmout", bufs=3,
                                     name="ot")
                        nc.vector.tensor_copy(ot[:, 0:n], pss[i][:, 0:n])
                        nc.sync.dma_start(
                            out_d[b * P:(b + 1) * P, vt * 512:vt * 512 + n],
                            ot[:, 0:n])

    nc.compile()
    return nc


# ---------------- host side ----------------

_CACHE = {}


def _perm_evenodd():
    p = []
    for h in range(NH):
        base = h * HD
        p += list(range(base, base + HD, 2))
        p += list(range(base + 1, base + HD, 2))
    return np.array(p)


def _prep_inputs(inputs):
    """Build per-core in_maps from full inputs."""
    ids_full = np.asarray(inputs["input_ids"]).astype(np.int32).reshape(SEQ)
    wte = np.asarray(inputs["wte"], dtype=np.float32)
    wqkv = np.asarray(inputs["wqkv"], dtype=np.float32)
    wo = np.asarray(inputs["wo"], dtype=np.float32)
    w1 = np.asarray(inputs["w1"], dtype=np.float32)
    w2 = np.asarray(inputs["w2"], dtype=np.float32)
    lm_head = np.asarray(inputs["lm_head"], dtype=np.float32)

    perm = _perm_evenodd()
    wqkvT = np.zeros((NLP, D, 3 * D), dtype=BF)
    woT = np.zeros((NLP, D, D), dtype=BF)
    w1T = np.zeros((NLP, D, FFN), dtype=BF)
    w2T = np.zeros((NLP, FFN, D), dtype=BF)
    for l in range(NL):
        wq = wqkv[l][:D][perm]
        wk_ = wqkv[l][D:2 * D][perm]
        wv = wqkv[l][2 * D:]
        wqkvT[l] = np.concatenate([wq, wk_, wv], axis=0).T.astype(BF)
        woT[l] = wo[l].T.astype(BF)
        w1T[l] = w1[l].T.astype(BF)
        w2T[l] = w2[l].T.astype(BF)

    inv_freq = 1.0 / (10000.0 ** (np.arange(0, HD, 2, dtype=np.float32) / HD))
    tril = np.tril(np.ones((P, P), np.float32))  # tril[q, k]: q >= k

    in_maps = []
    for c in range(NCORES):
        blocks = (c, 15 - c)
        ids_c = np.stack([ids_full[b * P:(b + 1) * P] for b in blocks], axis=1)
        pos = np.stack(
            [np.arange(b * P, (b + 1) * P, dtype=np.float32) for b in blocks],
            axis=1)  # [P, 2]
        ang = pos[:, :, None] * inv_freq[None, None, :]  # [P, 2, 32]

        g0, g1 = blocks
        # masks in scoresT orientation: mask[k_local, q_local] = 1 if valid
        mk07 = np.zeros((P, 8, 256), np.float32)
        for kb in range(8):
            if kb < g0:
                mk07[:, kb, 0:P] = 1.0
            elif kb == g0:
                mk07[:, kb, 0:P] = tril.T  # [k, q]: valid iff q >= k
            mk07[:, kb, P:256] = 1.0  # tile1 block g1 >= 8 > kb: always valid
        mk815 = np.zeros((P, 8, 128), np.float32)
        for kb in range(8, 16):
            if kb < g1:
                mk815[:, kb - 8] = 1.0
            elif kb == g1:
                mk815[:, kb - 8] = tril.T
        masks = np.concatenate(
            [mk07.reshape(P, 2048), mk815.reshape(P, 1024)], axis=1).astype(BF)

        lmT_c = np.zeros((D, VSP), dtype=BF)
        lmT_c[:, :VS] = lm_head[c * VS:(c + 1) * VS].T.astype(BF)

        in_maps.append({
            "ids": np.ascontiguousarray(ids_c),
            "wte": wte,
            "wqkvT": wqkvT,
            "woT": woT,
            "w1T": w1T,
            "w2T": w2T,
            "lmT": lmT_c,
            "cos": np.cos(ang).astype(np.float32),
            "sin": np.sin(ang).astype(np.float32),
            "masks": masks,
        })
    return in_maps


def run(inputs, trace=False, tmpdir=None):
    """Returns (full_logits [1, SEQ, VOCAB] f32, BassKernelResults)."""
    if "nc" not in _CACHE:
        _CACHE["nc"] = build_program()
    nc = _CACHE["nc"]
    in_maps = _prep_inputs(inputs)
    kw = {}
    if trace:
        kw = dict(trace=True)
        if tmpdir is not None:
            kw["tmpdir"] = tmpdir
    res = run_bass_kernel_spmd(nc, in_maps, core_ids=list(range(NCORES)), **kw)
    full = np.empty((SEQ, VOCAB), np.float32)
    for c in range(NCORES):
        full[:, c * VS:(c + 1) * VS] = res.results[c]["out"][:, :VS]
    return full.reshape(1, SEQ, VOCAB), res


def kernel(**inputs):
    out, _ = run(inputs)
    return out


def bench(inputs, iters=10, warmup=3):
    """Time device execution with device-resident inputs (no H2D in the loop).

    Returns (per_iter_seconds_list, outputs_core0_sample). Replicates
    bass2jax.run_bass_via_pjrt's multi-core shard_map structure, minus
    donation and minus host->device transfer inside the timed region.
    """
    import time as _time
    import jax
    from jax.experimental.shard_map import shard_map
    from jax.sharding import Mesh, PartitionSpec, NamedSharding
    from concourse import bass2jax as b2j
    from concourse import mybir as _mybir

    if "nc" not in _CACHE:
        _CACHE["nc"] = build_program()
    nc = _CACHE["nc"]
    in_maps = _prep_inputs(inputs)

    b2j.install_neuronx_cc_hook()
    partition_name = (nc.partition_id_tensor.name
                      if nc.partition_id_tensor else None)
    in_names, out_names, out_avals, zero_outs = [], [], [], []
    for alloc in nc.m.functions[0].allocations:
        if not isinstance(alloc, _mybir.MemoryLocationSet):
            continue
        name = alloc.memorylocations[0].name
        if alloc.kind == "ExternalInput":
            if name != partition_name:
                in_names.append(name)
        elif alloc.kind == "ExternalOutput":
            out_names.append(name)
            shape = tuple(alloc.tensor_shape)
            dtype = _mybir.dt.np(alloc.dtype)
            out_avals.append(jax.core.ShapedArray(shape, dtype))
            zero_outs.append(np.zeros(shape, dtype))
    n_params = len(in_names)
    all_in_names = list(in_names) + list(out_names)
    if partition_name is not None:
        all_in_names.append(partition_name)

    def _body(*args):
        operands = list(args)
        if partition_name is not None:
            operands.append(b2j.partition_id_tensor())
        outs = b2j._bass_exec_p.bind(
            *operands,
            out_avals=tuple(out_avals),
            in_names=tuple(all_in_names),
            out_names=tuple(out_names),
            lowering_input_output_aliases=(),
            sim_require_finite=True,
            sim_require_nnan=True,
            nc=nc,
        )
        return tuple(outs)

    devices = jax.devices()[:NCORES]
    mesh = Mesh(np.asarray(devices), ("core",))
    n_outs = len(out_names)
    in_specs = (PartitionSpec("core"),) * (n_params + n_outs)
    out_specs = (PartitionSpec("core"),) * n_outs
    fn = jax.jit(shard_map(_body, mesh=mesh, in_specs=in_specs,
                           out_specs=out_specs, check_rep=False))

    sharding = NamedSharding(mesh, PartitionSpec("core"))
    dev_args = []
    for i, name in enumerate(in_names):
        concat = np.concatenate([np.asarray(m[name]) for m in in_maps], axis=0)
        dev_args.append(jax.device_put(concat, sharding))
    for z in zero_outs:
        concat = np.zeros((NCORES * z.shape[0], *z.shape[1:]), z.dtype)
        dev_args.append(jax.device_put(concat, sharding))

    for _ in range(warmup):
        outs = fn(*dev_args)
        jax.block_until_ready(outs)
    times = []
    for _ in range(iters):
        t0 = _time.perf_counter()
        outs = fn(*dev_args)
        jax.block_until_ready(outs)
        times.append(_time.perf_counter() - t0)
    out0 = np.asarray(outs[0]).reshape(NCORES, *out_avals[0].shape)
    return times, out0

